# revision 16
# baseline (speedup 1.0000x reference)
"""Trainium2 Bass kernel for nn_Block_84602265797044 (gnn_message_passing).

Sharding: data-parallel over batch B=8 across 8 cores (1 batch item per core).
All params replicated. BatchNorm (training-mode) statistics are exchanged
with three tiny AllGathers (floor ~4.6us vs ~10us for AllReduce at 8 cores);
the 8-rank sum is recovered locally (tiny PE matmuls / partition_all_reduce).

Key algebraic simplifications (inherited from the validated baseline):
  * alt_mean = where(xi==xj, xj, 2 xj).mean(-1) equals 2*mean_j off-diagonal
    and mean_j on the diagonal.
  * The adjacency mask (pearson corr > 0) equals sign(Gram(l1 - rowmean)):
    BN1's affine scales Gram rows/cols by a_c^2 > 0, so the mask is computed
    before BN1 stats are known.
  * Softmax of sigmoid-bounded scores needs no max subtraction; masked
    softmax = (mask * exp(e2)) row-normalized.  Normalization is folded in
    via an all-ones column appended to l1N (row-sums appear as an extra
    output column of the attention matmuls).
  * att rows sum to 1, so att @ bn1(l1) = a_c * (att @ l1) + beta_c: BN1's
    affine is applied after attention, composed with BN2's affine.

Perf structure:
  * bf16 matmuls everywhere except the l1 path: the adjacency mask is
    sign(Gram(centered l1)) and bf16 noise there flips ~0.1% of mask bits
    (4x output error), so x -> l1 -> centering -> Gram stays f32.
  * Collective-dependent loads go on the ACT HWDGE ring (nc.scalar), bulk
    data on the SP ring (nc.sync): HWDGE executes FIFO per engine, so a
    load waiting on a collective must not head-of-line-block the shuffles.
  * Dummy identity matmuls ("warmers") run in collective shadows to keep
    the PE HAM clock-gate at 2.4 GHz for the following matmul phase.
  * Attention output is normalized, shuffled to [c,(n,d)] and BN2-statted
    in 4 pipelined c-chunks.
"""
import numpy as np

B, N, M, D, OUT, K = 8, 32, 64, 128, 128, 3
EPS = 1e-5
NCORES = 8
ND = N * D            # 4096
P2 = N * N            # 1024
R = P2 // 2           # 512
D2 = D + 2            # conv d-padded width

_cache = {}


def _bf16(a):
    from ml_dtypes import bfloat16
    return np.ascontiguousarray(np.asarray(a, np.float32).astype(bfloat16))


def build():
    import concourse.bacc as bacc
    import concourse.tile as tile
    from concourse import mybir, bass_isa

    f32 = mybir.dt.float32
    bf16 = mybir.dt.bfloat16
    AF = mybir.ActivationFunctionType
    OP = mybir.AluOpType
    AX = mybir.AxisListType
    RG = [list(range(NCORES))]

    nc = bacc.Bacc(None, target_bir_lowering=False)

    # ---------------- DRAM I/O ----------------
    xm_d = nc.dram_tensor("xm", [M, ND], f32, kind="ExternalInput")
    wl1t_d = nc.dram_tensor("wl1t", [M, M], f32, kind="ExternalInput")
    wsct_d = nc.dram_tensor("wsct", [M, OUT], bf16, kind="ExternalInput")
    wl3t_d = nc.dram_tensor("wl3t", [M, OUT], bf16, kind="ExternalInput")
    wfc_d = nc.dram_tensor("wfc", [128, 8 * R + 4 * P2], bf16, kind="ExternalInput")
    bands_d = nc.dram_tensor("bands", [M, N * 3 * M], bf16, kind="ExternalInput")
    identb_d = nc.dram_tensor("identb", [128, 128], bf16, kind="ExternalInput")
    identf_d = nc.dram_tensor("identf", [128, 128], f32, kind="ExternalInput")
    bnp_d = nc.dram_tensor("bnp", [128, 8], f32, kind="ExternalInput")
    bnpt_d = nc.dram_tensor("bnpt", [1, 128], f32, kind="ExternalInput")
    ones_d = nc.dram_tensor("ones", [128, 1], f32, kind="ExternalInput")
    v1_d = nc.dram_tensor("v1", [1, R], f32, kind="ExternalInput")
    out_d = nc.dram_tensor("outp", [OUT, ND], f32, kind="ExternalOutput")

    with tile.TileContext(nc) as tc:
        with tc.tile_pool(name="cst", bufs=1) as cst, \
             tc.tile_pool(name="big", bufs=1) as big, \
             tc.tile_pool(name="ps1", bufs=4, space="PSUM") as ps1, \
             tc.tile_pool(name="ps4", bufs=1, space="PSUM") as ps4, \
             tc.tile_pool(name="dram", bufs=1, space="DRAM") as dram:

            # ------------- load constants (small + first-needed first) ------
            identb = cst.tile([128, 128], bf16)
            nc.sync.dma_start(identb[:], identb_d[:])
            X = big.tile([M, ND], f32, tag="tagA")
            for q in range(4):
                nc.sync.dma_start(X[:, q * 1024:(q + 1) * 1024],
                                  xm_d[:, q * 1024:(q + 1) * 1024])
            wl1tf = cst.tile([M, M], f32)
            nc.scalar.dma_start(wl1tf[:], wl1t_d[:])
            wsct = cst.tile([M, OUT], bf16)
            nc.scalar.dma_start(wsct[:], wsct_d[:])
            wl3t = cst.tile([M, OUT], bf16)
            nc.scalar.dma_start(wl3t[:], wl3t_d[:])
            identf = cst.tile([128, 128], f32)
            nc.scalar.dma_start(identf[:], identf_d[:])
            bnp = cst.tile([128, 8], f32)
            nc.scalar.dma_start(bnp[:], bnp_d[:])
            bnpt = cst.tile([1, 128], f32)
            nc.scalar.dma_start(bnpt[:], bnpt_d[:])
            ones = cst.tile([128, 1], f32)
            nc.scalar.dma_start(ones[:], ones_d[:])
            wfc = big.tile([128, 8 * R + 4 * P2], bf16, tag="tagW")
            nc.sync.dma_start(wfc[:], wfc_d[:])
            w1tb = wfc[:, 0:8 * R].rearrange("p (q r) -> p q r", r=R)
            w2tb = wfc[:, 8 * R:].rearrange("p (q r) -> p q r", r=P2)
            bands = big.tile([M, N * 3, M], bf16, tag="tagBD")
            nc.sync.dma_start(bands[:].rearrange("p a b -> p (a b)"), bands_d[:])
            epst = cst.tile([128, 1], f32)
            nc.vector.memset(epst[:], EPS)
            # conv input d-padding (data-independent; done early, own slot)
            Y2p = big.tile([M, N * D2], bf16, tag="tagYP")
            nc.vector.memset(Y2p[:, 0:N * D2:D2], 0.0)
            nc.vector.memset(Y2p[:, D + 1:N * D2:D2], 0.0)

            # HAM warmers: keep/get the PE clock-gate open (no data deps
            # beyond identb; fill idle PE windows in program order)
            def warm(cnt):
                for _ in range(cnt):
                    pw = ps1.tile([128, 128], f32, tag="ps1")
                    nc.tensor.matmul(pw[:], identb[:], identb[:],
                                     start=True, stop=True)

            def warm_on(rhs_ap, cnt):
                # warmers gated on a late-arriving tile: they fire exactly in
                # the idle window after that tile is ready (e.g. a collective
                # wait), keeping the PE HAM clock-gate open for what follows
                rows = rhs_ap.shape[0]
                free = 1
                for dd in rhs_ap.shape[1:]:
                    free *= dd
                bp = rhs_ap.base_partition()
                for _ in range(cnt):
                    pw = ps1.tile([rows, free], f32, tag="ps1")
                    nc.tensor.matmul(pw[:], identb[bp:bp + rows, 0:rows],
                                     rhs_ap, start=True, stop=True)

            warm(12)   # during the X load

            # ------------- Ph1a: l1 = W_l1 @ x (f32) + stats -> AG1 ----------
            # l1 runs first so the (l1-only) stats AllGather launches early;
            # shortcut stats are only needed at the end and ride AG2 instead.
            l1f = big.tile([M, ND], f32, tag="tagB")
            Xb = big.tile([M, ND], bf16, tag="tagXB")
            sc = big.tile([OUT, ND], bf16, tag="tagF")
            l1s6 = cst.tile([M, 8, 6], f32)
            scs6 = cst.tile([OUT, 8, 6], f32)
            mi_l1 = cst.tile([M, N], f32)
            for k in range(8):
                pa = ps1.tile([M, 512], f32, tag="ps1")
                nc.tensor.matmul(pa[:], wl1tf[:], X[:, k * 512:(k + 1) * 512],
                                 start=True, stop=True)
                nc.scalar.copy(l1f[:, k * 512:(k + 1) * 512], pa[:])
                nc.vector.bn_stats(l1s6[:, k, :], pa[:])
                nc.vector.tensor_reduce(
                    mi_l1[:, 4 * k:4 * (k + 1)],
                    pa[:].rearrange("p (n d) -> p n d", d=D),
                    axis=AX.X, op=OP.add)
            l1ag = cst.tile([M, 2], f32)
            nc.vector.bn_aggr(l1ag[:], l1s6[:])
            ar1 = cst.tile([M, 2], f32)
            nc.scalar.copy(ar1[:, 0:1], l1ag[:, 0:1])
            t1 = cst.tile([M, 1], f32, tag="t1")
            nc.vector.tensor_mul(t1[:], l1ag[:, 0:1], l1ag[:, 0:1])
            nc.vector.tensor_add(ar1[:, 1:2], l1ag[:, 1:2], t1[:])
            c1i = dram.tile([M, 2], f32)
            c1o = dram.tile([NCORES, 128], f32)
            nc.scalar.dma_start(c1i[:], ar1[:])
            nc.gpsimd.collective_compute(
                "AllGather", OP.bypass, replica_groups=RG,
                ins=[c1i.opt()], outs=[c1o.opt()])

            # ------------- Ph1c: sc = W_sc @ x (bf16), in AG1's shadow ------
            for k in range(8):
                with tc.tile_wait_until(0.020):
                    nc.scalar.copy(Xb[:, k * 512:(k + 1) * 512],
                                   X[:, k * 512:(k + 1) * 512])
                pb = ps1.tile([OUT, 512], f32, tag="ps1")
                nc.tensor.matmul(pb[:], wsct[:], Xb[:, k * 512:(k + 1) * 512],
                                 start=True, stop=True)
                nc.vector.tensor_copy(sc[:, k * 512:(k + 1) * 512], pb[:])
                nc.vector.bn_stats(scs6[:, k, :], pb[:])
            scag = cst.tile([OUT, 2], f32)
            nc.vector.bn_aggr(scag[:], scs6[:])

            # ------------- Ph2: mask path (independent of AG1), all f32 -----
            # l1c = mi/D - l1  (negated centering; sign-irrelevant for Gram)
            l1cf = big.tile([M, ND], f32, tag="tagC")
            nc.vector.scalar_tensor_tensor(
                out=l1cf[:].rearrange("p (n d) -> p n d", d=D),
                in0=mi_l1[:].unsqueeze(2).broadcast_to((M, N, D)),
                scalar=1.0 / D,
                in1=l1f[:].rearrange("p (n d) -> p n d", d=D),
                op0=OP.mult, op1=OP.subtract)

            # transpose l1cf -> l1cT [d=128, (n, c)] f32
            l1cT = big.tile([128, N, M], f32, tag="tagT")
            for g in range(8):
                pt = ps1.tile([128, 4 * M], f32, tag="ps1")
                for u in range(4):
                    n = 4 * g + u
                    nc.tensor.transpose(pt[:, u * M:(u + 1) * M],
                                        l1cf[:, n * D:(n + 1) * D],
                                        identf[0:M, 0:M])
                nc.scalar.copy(
                    l1cT[:, 4 * g:4 * (g + 1), :].rearrange("p a b -> p (a b)"),
                    pt[:])

            # Gram per head -> mask01 [j, (c, i)] (0/1 bf16); symmetric in (i,j)
            mask01 = big.tile([N, M, N], bf16, tag="tagM1")
            for hf in range(2):
                psG = ps4.tile([N, M // 2, N], f32, tag="psG")
                for u in range(M // 2):
                    c = hf * (M // 2) + u
                    nc.tensor.matmul(psG[:, u, :], l1cT[:, :, c], l1cT[:, :, c],
                                     start=True, stop=True)
                nc.vector.tensor_scalar(
                    out=mask01[:, hf * (M // 2):(hf + 1) * (M // 2), :]
                        .rearrange("p a b -> p (a b)"),
                    in0=psG[:].rearrange("p a b -> p (a b)"),
                    scalar1=0.0, scalar2=None, op0=OP.is_gt)

            # ------------- Ph3: l1 -> l1N [n, (c, d+1)] via DRAM (chunked) ----
            # extra all-ones column folds softmax row-sums into the att matmuls
            dl1 = dram.tile([N, M, D], f32)
            l1v = l1f[:].rearrange("c (n d) -> c n d", d=D)
            for q in range(4):
                nc.sync.dma_start(
                    dl1[8 * q:8 * (q + 1), :, :].rearrange("n c d -> c n d"),
                    l1v[:, 8 * q:8 * (q + 1), :])
            l1N = big.tile([N, M, D + 1], bf16, tag="tagD")
            with tc.tile_wait_until(0.030):
                for q in range(4):
                    nc.gpsimd.dma_start(l1N[8 * q:8 * (q + 1), :, 0:D],
                                        dl1[8 * q:8 * (q + 1), :, :])
            nc.vector.memset(l1N[:, :, D:D + 1], 1.0)

            # ------------- AG1 recovery: 8-rank sum via tiny PE matmuls -----
            # (emitted after the mask path so the waiting matmuls do not
            #  head-of-line-block the PE queue)
            g1t = cst.tile([NCORES, 128], f32)
            with tc.tile_wait_until(0.048):
                nc.scalar.dma_start(g1t[:], c1o[:])
                pr1 = ps1.tile([M, 2], f32, tag="ps1")
                g1v = g1t[:].rearrange("p (c f) -> p c f", f=2)
                for f in range(2):
                    nc.tensor.matmul(pr1[:, f:f + 1], g1v[:, :, f],
                                     ones[0:NCORES, 0:1], start=True, stop=True)
                ar1r = cst.tile([M, 2], f32)
                nc.vector.tensor_copy(ar1r[:], pr1[:])

            # post-AG affines: a = g / sqrt(var+eps), beta = b - a*mean
            def bn_affine(mean_col, e2_col, gcol, bcol, av, bv, nrows):
                tm = cst.tile([128, 1], f32, tag="tm")
                te = cst.tile([128, 1], f32, tag="te")
                nc.scalar.mul(tm[:nrows, :], mean_col, 1.0 / NCORES)
                nc.scalar.mul(te[:nrows, :], e2_col, 1.0 / NCORES)
                tv = cst.tile([128, 1], f32, tag="tv")
                nc.vector.tensor_mul(tv[:nrows, :], tm[:nrows, :], tm[:nrows, :])
                nc.vector.tensor_sub(te[:nrows, :], te[:nrows, :], tv[:nrows, :])
                nc.scalar.activation(te[:nrows, :], te[:nrows, :], AF.Sqrt,
                                     bias=epst[:nrows, :])
                nc.vector.reciprocal(te[:nrows, :], te[:nrows, :])
                nc.vector.tensor_mul(av[:nrows, :], gcol, te[:nrows, :])
                nc.vector.tensor_mul(tv[:nrows, :], av[:nrows, :], tm[:nrows, :])
                nc.vector.tensor_sub(bv[:nrows, :], bcol, tv[:nrows, :])

            a1v = cst.tile([128, 1], f32)
            b1v = cst.tile([128, 1], f32)
            bn_affine(ar1r[:, 0:1], ar1r[:, 1:2], bnp[0:M, 0:1], bnp[0:M, 1:2],
                      a1v, b1v, M)

            # ------------- Ph4: e -> fc1 -> fc2 -> pA [c, (i,j)] -------------
            a1s = cst.tile([128, 1], f32)
            nc.scalar.mul(a1s[0:M, :], a1v[0:M, :], 1.0 / D)
            mi = cst.tile([M, N], f32)
            nc.scalar.activation(mi[:], mi_l1[:], AF.Identity,
                                 bias=b1v[0:M, :], scale=a1s[0:M, :])
            mih = cst.tile([M, N], f32)
            nc.scalar.mul(mih[:], mi[:], 0.5)
            e = big.tile([M, P2], bf16, tag="tagEH")
            nc.vector.tensor_tensor(
                out=e[:].rearrange("p (i j) -> p i j", j=N),
                in0=mih[:].unsqueeze(2).broadcast_to((M, N, N)),
                in1=mi[:].unsqueeze(1).broadcast_to((M, N, N)),
                op=OP.add)
            nc.vector.tensor_sub(e[:, 0:P2:N + 1], e[:, 0:P2:N + 1], mih[:])
            ebT = cst.tile([128, 8, M], bf16)
            for q in range(8):
                pt2 = ps1.tile([128, M], bf16, tag="ps1")
                nc.tensor.transpose(pt2[:], e[:, q * 128:(q + 1) * 128],
                                    identb[0:M, 0:M])
                nc.scalar.copy(ebT[:, q, :], pt2[:])
            ph = ps1.tile([M, R], f32, tag="ps1")
            for q in range(8):
                nc.tensor.matmul(ph[:], ebT[:, q, :], w1tb[:, q, :],
                                 start=(q == 0), stop=(q == 7))
            h = big.tile([M, R], bf16, tag="tagEH")  # e dead after eT
            nc.scalar.activation(h[:], ph[:], AF.Relu)

            hbT = cst.tile([128, 4, M], bf16)
            for q in range(4):
                pt3 = ps1.tile([128, M], bf16, tag="ps1")
                nc.tensor.transpose(pt3[:], h[:, q * 128:(q + 1) * 128],
                                    identb[0:M, 0:M])
                nc.scalar.copy(hbT[:, q, :], pt3[:])

            # fc2: z = W2 @ h -> [c, p]; pA = exp(sigmoid(z))
            pz = ps4.tile([M, P2], f32, tag="pz")
            for half in range(2):
                for q in range(4):
                    nc.tensor.matmul(pz[:, half * 512:(half + 1) * 512],
                                     hbT[:, q, :],
                                     w2tb[:, q, half * 512:(half + 1) * 512],
                                     start=(q == 0), stop=(q == 3))
            pA = big.tile([M, P2], bf16, tag="tagT")  # l1cT dead after Gram
            nc.scalar.activation(pA[:], pz[:], AF.Sigmoid)
            nc.scalar.activation(pA[:], pA[:], AF.Exp)

            warm_on(pA[:, 0:512], 16)   # keep PE warm through the DVE
                                        # transposes for the head matmuls

            # ------------- Ph5: DVE transpose halves -> pBT [j, (i, c)] ------
            pBTa = big.tile([N, N, 32], bf16, tag="tagPA")
            pBTb = big.tile([N, N, 32], bf16, tag="tagPB")
            nc.vector.transpose(pBTa[:].rearrange("p a b -> p (a b)"),
                                pA[0:32, :])
            nc.vector.transpose(pBTb[:].rearrange("p a b -> p (a b)"),
                                pA[32:64, :])
            # mask multiply (symmetric mask viewed [j, i, c])
            nc.vector.tensor_tensor(
                out=pBTa[:], in0=pBTa[:],
                in1=mask01[:, 0:32, :].rearrange("p c i -> p i c"),
                op=OP.mult)
            nc.vector.tensor_tensor(
                out=pBTb[:], in0=pBTb[:],
                in1=mask01[:, 32:64, :].rearrange("p c i -> p i c"),
                op=OP.mult)
            # ------------- Ph6/7: attention matmuls + normalize + shuffle ----
            Yn = big.tile([N, M, D + 1], bf16, tag="tagE")
            rr = cst.tile([N, M], f32)
            dy = dram.tile([M, N, D], bf16)
            Yc = big.tile([M, N, D], bf16, tag="tagC")  # l1cf dead
            ys6 = cst.tile([M, 8, 6], f32)
            for q in range(4):
                for g in range(8):
                    py = ps1.tile([N, 2, D + 1], f32, tag="ps1")
                    for u in range(2):
                        c = 16 * q + 2 * g + u
                        src = pBTa if c < 32 else pBTb
                        nc.tensor.matmul(py[:, u, :], src[:, :, c % 32],
                                         l1N[:, c, :], start=True, stop=True)
                    dst = Yn[:, 16 * q + 2 * g:16 * q + 2 * (g + 1), :] \
                        .rearrange("p a b -> p (a b)")
                    if g % 2 == 0:
                        nc.scalar.copy(dst, py[:].rearrange("p a b -> p (a b)"))
                    else:
                        nc.vector.tensor_copy(
                            dst, py[:].rearrange("p a b -> p (a b)"))
                cq = slice(16 * q, 16 * (q + 1))
                nc.vector.reciprocal(rr[:, cq], Yn[:, cq, D])
                nc.vector.tensor_tensor(
                    out=Yn[:, cq, 0:D], in0=Yn[:, cq, 0:D],
                    in1=rr[:, cq].unsqueeze(2).broadcast_to((N, 16, D)),
                    op=OP.mult)
                nc.sync.dma_start(
                    dy[cq, :, :].rearrange("c n d -> n c d"),
                    Yn[:, cq, 0:D])
                nc.sync.dma_start(Yc[cq, :, :], dy[cq, :, :])
                if q % 2 == 1:  # BNStats needs 32-aligned partition starts
                    ch = slice(32 * (q // 2), 32 * (q // 2) + 32)
                    Ych = Yc[ch, :, :].rearrange("p a b -> p (a b)")
                    for g in range(8):
                        nc.vector.bn_stats(ys6[ch, g, :],
                                           Ych[:, g * 512:(g + 1) * 512])

            # ------------- Ph8: BN2 + BNsc aggregation + AG2 -------------
            yag = cst.tile([M, 2], f32)
            nc.vector.bn_aggr(yag[:], ys6[:])
            ar2 = cst.tile([128, 4], f32)
            nc.vector.memset(ar2[:], 0.0)
            # cols: 0=mean_Y, 1=E2_Y, 2=mean_sc, 3=E2_sc
            nc.scalar.copy(ar2[0:M, 0:1], yag[:, 0:1])
            t2 = cst.tile([128, 1], f32, tag="t2")
            nc.vector.tensor_mul(t2[0:M, :], yag[:, 0:1], yag[:, 0:1])
            nc.vector.tensor_add(ar2[0:M, 1:2], yag[:, 1:2], t2[0:M, :])
            nc.scalar.copy(ar2[:, 2:3], scag[:, 0:1])
            nc.vector.tensor_mul(t2[:], scag[:, 0:1], scag[:, 0:1])
            nc.vector.tensor_add(ar2[:, 3:4], scag[:, 1:2], t2[:])
            c2i = dram.tile([128, 4], f32)
            c2o = dram.tile([NCORES, 512], f32)
            nc.scalar.dma_start(c2i[:], ar2[:])
            nc.gpsimd.collective_compute(
                "AllGather", OP.bypass, replica_groups=RG,
                ins=[c2i.opt()], outs=[c2o.opt()])

            # warmers for the conv phase, gated on the last Yc chunk so they
            # fire inside the AG2 wait window
            with tc.tile_wait_until(0.115):
                warm_on(Yc[32:64, 31, :], 80)

            g2t = cst.tile([NCORES, 512], f32)
            with tc.tile_wait_until(0.125):
                nc.scalar.dma_start(g2t[:], c2o[:])
                pr2 = ps1.tile([128, 4], f32, tag="ps1")
                g2v = g2t[:].rearrange("p (c f) -> p c f", f=4)
                for f in range(4):
                    nc.tensor.matmul(pr2[:, f:f + 1], g2v[:, :, f],
                                     ones[0:NCORES, 0:1], start=True, stop=True)
                ar2r = cst.tile([128, 4], f32)
                nc.vector.tensor_copy(ar2r[:], pr2[:])
            asc = cst.tile([128, 1], f32)
            bsc = cst.tile([128, 1], f32)
            bn_affine(ar2r[:, 2:3], ar2r[:, 3:4], bnp[:, 6:7], bnp[:, 7:8],
                      asc, bsc, 128)

            # composite affine: Y2 = relu(A * Y_raw + Bv)
            mr = cst.tile([M, 1], f32, tag="mr")
            e2r = cst.tile([M, 1], f32, tag="e2r")
            nc.scalar.mul(mr[:], ar2r[0:M, 0:1], 1.0 / NCORES)
            nc.scalar.mul(e2r[:], ar2r[0:M, 1:2], 1.0 / NCORES)
            vr = cst.tile([M, 1], f32, tag="vr")
            nc.vector.tensor_mul(vr[:], mr[:], mr[:])
            nc.vector.tensor_sub(vr[:], e2r[:], vr[:])         # var_raw
            a1sq = cst.tile([M, 1], f32, tag="a1sq")
            nc.vector.tensor_mul(a1sq[:], a1v[0:M, :], a1v[0:M, :])
            nc.vector.tensor_mul(vr[:], vr[:], a1sq[:])        # var_final
            nc.scalar.activation(vr[:], vr[:], AF.Sqrt, bias=epst[0:M, :])
            nc.vector.reciprocal(vr[:], vr[:])
            a2 = cst.tile([M, 1], f32, tag="a2")
            nc.vector.tensor_mul(a2[:], bnp[0:M, 2:3], vr[:])
            Av = cst.tile([M, 1], f32, tag="Av")
            nc.vector.tensor_mul(Av[:], a2[:], a1v[0:M, :])
            Bv = cst.tile([M, 1], f32, tag="Bv")
            nc.vector.tensor_mul(Bv[:], Av[:], mr[:])
            nc.vector.tensor_sub(Bv[:], bnp[0:M, 3:4], Bv[:])

            Y2pv = Y2p[:].rearrange("p (n d) -> p n d", d=D2)
            nc.scalar.activation(Y2pv[:, 0:N:2, 1:D + 1], Yc[:, 0:N:2, :],
                                 AF.Relu, bias=Bv[:], scale=Av[:])
            Y2po = Y2pv[:, 1:N:2, 1:D + 1]
            Avb = Av[:].broadcast_to((M, N // 2)).unsqueeze(2) \
                .broadcast_to((M, N // 2, D))
            Bvb = Bv[:].broadcast_to((M, N // 2)).unsqueeze(2) \
                .broadcast_to((M, N // 2, D))
            nc.vector.tensor_tensor(out=Y2po, in0=Yc[:, 1:N:2, :], in1=Avb,
                                    op=OP.mult)
            nc.vector.tensor_tensor(out=Y2po, in0=Y2po, in1=Bvb, op=OP.add)
            nc.vector.tensor_scalar(out=Y2po, in0=Y2po, scalar1=0.0,
                                    scalar2=None, op0=OP.max)

            # ------------- Ph9: depthwise conv + BN3 stats -------------
            convb = big.tile([M, N, D], bf16, tag="tagA")  # X dead
            cs6 = cst.tile([M, N, 6], f32)
            for n in range(N):
                pc = ps1.tile([M, D], f32, tag="ps1")
                for kw in range(3):
                    nc.tensor.matmul(
                        pc[:], bands[:, n * 3 + kw, :],
                        Y2p[:, n * D2 + kw: n * D2 + kw + D],
                        start=(kw == 0), stop=(kw == 2))
                nc.vector.bn_stats(cs6[:, n, :], pc[:])
                if n % 2 == 0:
                    nc.scalar.copy(convb[:, n, :], pc[:])
                else:
                    nc.vector.tensor_copy(convb[:, n, :], pc[:])

            # BN3 per-n stats: st cols 0..31 mean, 32..63 E2
            st = cst.tile([M, 2 * N], f32)
            me = cs6[:, :, 1:2].rearrange("p a b -> p (a b)")
            mo = cs6[:, :, 4:5].rearrange("p a b -> p (a b)")
            nc.vector.tensor_add(st[:, 0:N], me, mo)
            nc.scalar.mul(st[:, 0:N], st[:, 0:N], 0.5)
            tm2 = cst.tile([M, N], f32, tag="tm2")
            tm3 = cst.tile([M, N], f32, tag="tm3")
            nc.vector.tensor_mul(tm2[:], me, me)
            nc.vector.tensor_mul(tm3[:], mo, mo)
            nc.vector.tensor_add(tm2[:], tm2[:], tm3[:])
            nc.vector.tensor_add(
                tm3[:], cs6[:, :, 2:3].rearrange("p a b -> p (a b)"),
                cs6[:, :, 5:6].rearrange("p a b -> p (a b)"))
            nc.scalar.mul(tm3[:], tm3[:], 1.0 / 64.0)
            nc.vector.tensor_add(tm2[:], tm2[:], tm3[:])
            nc.scalar.mul(st[:, N:2 * N], tm2[:], 0.5)
            ps3 = ps1.tile([1, 2 * N], f32, tag="ps1")
            nc.tensor.matmul(ps3[:], ones[0:M, :], st[:], start=True, stop=True)
            ar3 = cst.tile([1, 2 * N], f32)
            nc.scalar.copy(ar3[:], ps3[:])
            c3i = dram.tile([1, 2 * N], f32)
            c3o = dram.tile([NCORES, 2 * N], f32)
            nc.scalar.dma_start(c3i[:], ar3[:])
            nc.gpsimd.collective_compute(
                "AllGather", OP.bypass, replica_groups=RG,
                ins=[c3i.opt()], outs=[c3o.opt()])

            # warmers for Y3/l3, gated on the last conv column; the shortcut
            # affine also runs here (scalar engine, AG3 shadow)
            with tc.tile_wait_until(0.160):
                warm_on(convb[:, 28:32, :].rearrange("p a b -> p (a b)"), 56)
                nc.scalar.activation(sc[:], sc[:], AF.Identity,
                                     bias=bsc[:], scale=asc[:])

            g3t = cst.tile([NCORES, 2 * N], f32)
            with tc.tile_wait_until(0.170):
                nc.scalar.dma_start(g3t[:], c3o[:])
                g3s = cst.tile([NCORES, 2 * N], f32)
                nc.gpsimd.partition_all_reduce(g3s[:], g3t[:], channels=NCORES,
                                               reduce_op=bass_isa.ReduceOp.add)
            ar3r = g3s[0:1, :]

            # affine per n; g3 = bnpt[64:96], b3 = bnpt[96:128]
            m3 = cst.tile([1, N], f32, tag="m3")
            nc.scalar.mul(m3[:], ar3r[:, 0:N], 1.0 / (M * NCORES))
            E3 = cst.tile([1, N], f32, tag="E3")
            nc.scalar.mul(E3[:], ar3r[:, N:2 * N], 1.0 / (M * NCORES))
            v3 = cst.tile([1, N], f32, tag="v3")
            nc.vector.tensor_mul(v3[:], m3[:], m3[:])
            nc.vector.tensor_sub(v3[:], E3[:], v3[:])
            nc.scalar.activation(v3[:], v3[:], AF.Sqrt, bias=epst[0:1, :])
            nc.vector.reciprocal(v3[:], v3[:])
            a3r = cst.tile([1, 2 * N], f32)    # [a3 | beta3]
            nc.vector.tensor_mul(a3r[:, 0:N], bnpt[:, 64:64 + N], v3[:])
            nc.vector.tensor_mul(v3[:], a3r[:, 0:N], m3[:])
            nc.vector.tensor_sub(a3r[:, N:2 * N], bnpt[:, 96:96 + N], v3[:])
            ab3 = cst.tile([M, 2 * N], f32)
            nc.gpsimd.partition_broadcast(ab3[:], a3r[0:1, :])

            # ------------- Ph10: bn3+relu, l3, +shortcut, out -------------
            # even n on the scalar engine (per-n ACT), odd n in 3 wide DVE ops
            Y3 = big.tile([M, ND], bf16, tag="tagXB")  # Xb dead
            NS = 8   # first NS columns via scalar ACT (gates l3 k=0,1), the
            for n in range(NS):  # rest in one contiguous 3-op DVE chain
                nc.scalar.activation(
                    Y3[:, n * D:(n + 1) * D],
                    convb[:, n, :], AF.Relu,
                    bias=ab3[:, N + n:N + n + 1], scale=ab3[:, n:n + 1])
            Y3v = Y3[:].rearrange("p (n d) -> p n d", d=D)[:, NS:N, :]
            cvo = convb[:, NS:N, :]
            a3o = ab3[:, NS:N].unsqueeze(2).broadcast_to((M, N - NS, D))
            b3o = ab3[:, N + NS:2 * N].unsqueeze(2).broadcast_to((M, N - NS, D))
            nc.vector.tensor_tensor(out=Y3v, in0=cvo, in1=a3o, op=OP.mult)
            nc.vector.tensor_tensor(out=Y3v, in0=Y3v, in1=b3o, op=OP.add)
            nc.vector.tensor_scalar(out=Y3v, in0=Y3v, scalar1=0.0,
                                    scalar2=None, op0=OP.max)

            outsb = big.tile([OUT, ND], f32, tag="tagD")  # l1N dead
            for k in range(8):
                pl = ps1.tile([OUT, 512], f32, tag="ps1")
                nc.tensor.matmul(pl[:], wl3t[:], Y3[:, k * 512:(k + 1) * 512],
                                 start=True, stop=False)
                nc.tensor.matmul(pl[:], identb[:],
                                 sc[:, k * 512:(k + 1) * 512],
                                 start=False, stop=True)
                if k % 2 == 0:
                    nc.scalar.copy(outsb[:, k * 512:(k + 1) * 512], pl[:])
                else:
                    nc.vector.tensor_copy(outsb[:, k * 512:(k + 1) * 512],
                                          pl[:])
                nc.sync.dma_start(out_d[:, k * 512:(k + 1) * 512],
                                  outsb[:, k * 512:(k + 1) * 512])
    nc.finalize()
    return nc


def _prep_inputs(x, W_sc, g_sc, b_sc, W_l1, g1, b1, W_fc1, W_fc2, g2, b2,
                 W_dw, g3, b3, W_l3):
    f = np.float32
    xm = np.ascontiguousarray(np.transpose(x, (0, 2, 1, 3)), dtype=f)  # (B,M,N,D)
    wl1t = np.ascontiguousarray(W_l1.T, dtype=f)
    wsct = _bf16(W_sc.T)
    wl3t = _bf16(W_l3.T)
    w1tb = W_fc1.T.reshape(8, 128, R).transpose(1, 0, 2).reshape(128, 8 * R)
    w2tb = W_fc2.T.reshape(4, 128, P2).transpose(1, 0, 2).reshape(128, 4 * P2)
    wfc = _bf16(np.concatenate([w1tb, w2tb], axis=1))
    band = np.zeros((N, 3, M, M), f)
    for kh in range(3):
        for kw in range(3):
            for m in range(M):
                p = m + kh - 1
                if 0 <= p < M:
                    band[:, kw, p, m] = W_dw[:, 0, kh, kw]
    bands = _bf16(band.transpose(2, 0, 1, 3).reshape(M, N * 3 * M))
    identb = _bf16(np.eye(128, dtype=f))
    identf = np.eye(128, dtype=f)
    bnp = np.zeros((128, 8), f)
    bnp[:M, 0] = g1; bnp[:M, 1] = b1
    bnp[:M, 2] = g2; bnp[:M, 3] = b2
    bnp[:N, 4] = g3; bnp[:N, 5] = b3
    bnp[:, 6] = g_sc; bnp[:, 7] = b_sc
    bnpt = np.zeros((1, 128), f)
    bnpt[0, 64:64 + N] = g3
    bnpt[0, 96:96 + N] = b3
    ones = np.ones((128, 1), f)
    pat = np.full((N, N), 1.5, f)
    np.fill_diagonal(pat, 1.0)
    v1 = (np.asarray(W_fc1, f) @ pat.reshape(-1)).reshape(1, R)
    shared = dict(wl1t=wl1t, wsct=wsct, wl3t=wl3t, wfc=wfc,
                  bands=bands, identb=identb, identf=identf, bnp=bnp,
                  bnpt=bnpt, ones=ones, v1=np.ascontiguousarray(v1))
    in_maps = []
    for b in range(B):
        m = dict(shared)
        m["xm"] = np.ascontiguousarray(xm[b].reshape(M, ND))
        in_maps.append(m)
    return in_maps


def _run(inputs, trace=False, debug=False, tmpdir=None):
    from concourse import bass_utils
    if "nc" not in _cache:
        _cache["nc"] = build()
    nc = _cache["nc"]
    in_maps = _prep_inputs(**inputs)
    res = bass_utils.run_bass_kernel_spmd(
        nc, in_maps, core_ids=list(range(NCORES)), trace=trace, tmpdir=tmpdir)
    outs = []
    for b in range(B):
        o = res.results[b]["outp"].reshape(OUT, N, D).transpose(1, 0, 2)
        outs.append(o)
    full = np.stack(outs).astype(np.float32)  # (B, N, OUT, D)
    return full, res


def kernel(**inputs):
    full, _ = _run(inputs, trace=False)
    return full
